# revision 38
# baseline (speedup 1.0000x reference)
"""Multi-head attention (QKV proj + rotary + softmax attention + out proj)
for Trainium2, sharded over 8 NeuronCores.

Problem: x[2,2048,1024], 16 heads x dh=64, rotary embedding, softmax
attention, output projection + bias.

Sharding: batch x head-group. Core c handles batch c//4 and the 4 heads
[4*(c%4), 4*(c%4)+4). Each core computes its QKV slice, rotary, attention,
and a partial output projection; the host sums the 4 partial projections
per batch and adds the bias.

Device-side design (per core, everything in "transposed" layout):
  - all inputs (x, weights, cos/sin) converted to bf16 on the host: halves
    HBM traffic and keeps every matmul at the full 1-cycle/row PE rate.
    Measured rel err ~7e-3 (gate 2e-2).
  - qkvT = W @ x^T as bf16 matmuls: qT/kT produced as [dh-pair(128), n]
    tiles, v as natural [n, e] tiles, one head-pair at a time.
  - rotary applied on the fp32 psum output via DVE: q*cos +
    pairswap(q*sin_pre), with the dh dimension stored interleaved
    ([0,32,1,33,...]) so rotate_half becomes an adjacent-lane
    stream_shuffle. Output bf16.
  - dots: scoresT[j,n] = krotT^T-slice @ qrotT, two heads packed in the
    128x128 PE array via tile_position row-tiling (K=64 each, concurrent).
  - softmax without max-subtraction (logits are O(+-6)): ACT exp over
    2-j-tile psum batches (N=1024 per ACTIVATE), output fp16. The exp
    stream (128 ACTIVATEs, ~142us) is the kernel's critical resource;
    everything else is scheduled to hide under it.
  - AV: lhsT = [ones | zeros(63) | v(64)] (M=128, fp16) so psum row 0
    accumulates the softmax denominators (base_partition 0 for the DVE
    reciprocal) and values land at rows 64-127 (32-aligned); fp32 psum
    accumulation over the 16 j-tiles, one aligned evacuation copy.
  - normalize: reciprocal_approx_fast of the sums row, partition-broadcast
    via a DRAM round-trip DMA (K=1 ones-matmul on the tail block), one DVE
    multiply -> aoT (bf16).
  - output proj: y[n,d] accumulated over the two head-pair e-chunks, fp16
    out; host sums the per-core partials in fp32 and adds the bias.

Scheduling (the measured-trace-driven part):
  - DMA-completion processing serializes at ~0.67us/DMA on the Sync
    engine, so the preamble uses few, need-ordered dma_starts: first-needed
    tiles small and early, later ones fat. All of x (bf16, 4MB) is
    prefetched once and stays resident.
  - the head interleaves the k0/q0 psum chains chunk-wise so the first
    dots fire as soon as the last x chunk's completion is visible.
  - per-tile k projections, next-tile q rotations and per-128-row y
    projections are threaded through the attention j-loops one chunk per
    j-batch with >=1 batch of slack before their consumer, so the dots
    (which gate exp) never queue behind a lump of projection matmuls in
    the PE FIFO. Block (0,0) computes only pair-0's k/v; block (0,1)
    computes pair-1's, balancing both warmup blocks against the exp
    stream and keeping the PE HAM window at K=8/8 throughout.
"""
import sys

sys.path.insert(0, "/opt/trn_rl_repo")

import numpy as np

import concourse.bacc as bacc
import concourse.tile as tile
from concourse import mybir
from concourse.bass_utils import run_bass_kernel_spmd

F32 = mybir.dt.float32
F32R = mybir.dt.float32r
BF16 = mybir.dt.bfloat16
FP16 = mybir.dt.float16
EXP = mybir.ActivationFunctionType.Exp
MULT = mybir.AluOpType.mult
ADD = mybir.AluOpType.add

B, N, DIM = 2, 2048, 1024
H, DH = 16, 64
INNER = H * DH
SCALE = DH ** -0.5
NCORES = 8
HPC = H // (NCORES // B)      # heads per core = 4
NPAIR = HPC // 2              # head pairs per core = 2

P = 128
NT = N // 512                 # 4 n-tiles of 512
DC = DIM // P                 # 8 d-chunks
JTILES = N // P               # 16 j-tiles
JB = JTILES // 2              # 8 j-batches (2 j-tiles each)

PAIRSWAP = [i ^ 1 for i in range(32)]

_CACHE = {}


def _build():
    nc = bacc.Bacc(None, target_bir_lowering=False, debug=False)
    with tile.TileContext(nc) as tc:
        with tc.tile_pool(name="dram", bufs=1, space="DRAM") as dram, \
             tc.tile_pool(name="const", bufs=1) as const, \
             tc.tile_pool(name="perst", bufs=1) as perst, \
             tc.tile_pool(name="tmp", bufs=1) as tmp, \
             tc.tile_pool(name="ps", bufs=1, space="PSUM") as ps:
            # ---------------- DRAM I/O ----------------
            xT_d = dram.tile([DIM, N], BF16, kind="ExternalInput", name="xT", uniquify=False)
            wqkT_d = dram.tile([DIM, 512], BF16, kind="ExternalInput", name="wqkT", uniquify=False)
            wvT_d = dram.tile([DIM, 256], BF16, kind="ExternalInput", name="wvT", uniquify=False)
            cq_d = dram.tile([P, N], BF16, kind="ExternalInput", name="cq", uniquify=False)
            sq_d = dram.tile([P, N], BF16, kind="ExternalInput", name="sq", uniquify=False)
            ck_d = dram.tile([P, N], BF16, kind="ExternalInput", name="ck", uniquify=False)
            sk_d = dram.tile([P, N], BF16, kind="ExternalInput", name="sk", uniquify=False)
            woT_d = dram.tile([256, DIM], BF16, kind="ExternalInput", name="woT", uniquify=False)
            y_d = dram.tile([N, DIM], FP16, kind="ExternalOutput", name="y", uniquify=False)
            y3a_d = dram.tile([512, DIM], FP16, kind="ExternalOutput", name="y3a", uniquify=False)

            xT_r = xT_d.rearrange("(c p) n -> p c n", p=P)
            wqk_r = wqkT_d.rearrange("(c p) e -> p c e", p=P)
            wv_r = wvT_d.rearrange("(c p) e -> p c e", p=P)
            wo_r = woT_d.rearrange("(c p) d -> p c d", p=P)

            # ------------- const tiles -------------
            # DMA-completion processing serializes at ~0.67us each on the Sync
            # engine, so the preamble uses as FEW dma_starts as data-arrival
            # deadlines allow: first-needed tiles small and early, the rest fat.
            # e-chunk order in wqkT columns: q0=0, q1=1, k0=2, k1=3 (x128)
            wqk_sb = [[const.tile([P, 4, P], BF16, name=f"wqk{e}_{hf}")
                       for hf in range(2)] for e in range(4)]
            wv_sb = [const.tile([P, 4, 256], BF16, name=f"wv{hf}")
                     for hf in range(2)]
            wo_sb = [[const.tile([P, 512], BF16, name=f"wo{p}_{d}")
                      for d in range(2)] for p in range(NPAIR)]
            cqt = [const.tile([P, 512], BF16, name=f"cqt{t}") for t in range(NT)]
            sqt = [const.tile([P, 512], BF16, name=f"sqt{t}") for t in range(NT)]
            ckt = [const.tile([P, 512], BF16, name=f"ckt{t}") for t in range(NT)]
            skt = [const.tile([P, 512], BF16, name=f"skt{t}") for t in range(NT)]

            def wqk_chunk(e, c):
                return wqk_sb[e][c // 4][:, c % 4, :]

            def dma_wqk(e, hf):
                nc.sync.dma_start(
                    wqk_sb[e][hf][:, :, :],
                    wqk_r[:, hf * 4:(hf + 1) * 4, e * P:(e + 1) * P])

            def dma_cs(tiles, src_d, t):
                nc.sync.dma_start(tiles[t][:, :], src_d[:, t * 512:(t + 1) * 512])

            # all x tiles prefetched in the preamble (bf16: 4MB total) and
            # kept resident for the whole kernel -- x is loaded exactly once.
            # t0 is chunk-granular (feeds the very first matmuls); t1-3 use
            # c-pair tiles (half the dma_starts, deadlines are later).
            xt0 = [tmp.tile([P, 512], BF16, name=f"xt0_{c}", tag=f"xt{c}",
                            bufs=1) for c in range(DC)]
            xtr = [[tmp.tile([P, 2, 512], BF16, name=f"xt{t}_{cp}",
                             tag=f"xtp{cp}", bufs=3) for cp in range(DC // 2)]
                   for t in range(1, NT)]

            def x_chunk(t, c):
                if t == 0:
                    return xt0[c][:, :]
                return xtr[t - 1][c // 2][:, c % 2, :]

            def dma_x(t):
                for cp in range(DC // 2):
                    nc.sync.dma_start(
                        xtr[t - 1][cp][:, :, :],
                        xT_r[:, 2 * cp:2 * cp + 2, t * 512:(t + 1) * 512])

            # ---------------- preamble emission (need-order) ----------------
            for hf in range(2):
                dma_wqk(2, hf)         # k0
            for c in range(DC):
                nc.sync.dma_start(xt0[c][:, :], xT_r[:, c, 0:512])
            dma_cs(ckt, ck_d, 0)
            dma_cs(skt, sk_d, 0)
            for hf in range(2):
                dma_wqk(0, hf)         # q0
            dma_cs(cqt, cq_d, 0)
            dma_cs(sqt, sq_d, 0)
            dma_cs(ckt, ck_d, 1)
            dma_cs(skt, sk_d, 1)
            dma_x(1)
            for hf in range(2):
                nc.sync.dma_start(wv_sb[hf][:, :, :],
                                  wv_r[:, hf * 4:(hf + 1) * 4, :])
            for hf in range(2):
                dma_wqk(3, hf)         # k1
            dma_cs(ckt, ck_d, 2)
            dma_cs(skt, sk_d, 2)
            for hf in range(2):
                dma_wqk(1, hf)         # q1
            dma_cs(ckt, ck_d, 3)
            dma_cs(skt, sk_d, 3)
            dma_x(2)
            dma_x(3)
            for t in range(1, NT):
                dma_cs(cqt, cq_d, t)
                dma_cs(sqt, sq_d, t)
            for p in range(NPAIR):
                for d in range(2):
                    nc.sync.dma_start(wo_sb[p][d][:, :], wo_r[:, p, d * 512:(d + 1) * 512])

            ones_f = const.tile([1, 64], F32)
            nc.vector.memset(ones_f[:, :], 1.0)
            ones_r = const.tile([1, 64], BF16)
            nc.vector.tensor_copy(ones_r[:, :], ones_f[:, :])

            # ---------------- persistent tiles ----------------
            qrot = [[perst.tile([P, 512], BF16, name=f"qrot{p}_{t}")
                     for t in range(NT)] for p in range(NPAIR)]
            krot = [[perst.tile([P, 512], BF16, name=f"krot{p}_{t}")
                     for t in range(NT)] for p in range(NPAIR)]
            # AV stationary operand is [ones | zeros(63) | v(64)] so psum row 0
            # accumulates the softmax denominators (base_partition 0 for the
            # DVE reciprocal) and the values land at rows 64-127 (32-aligned
            # partition base) -- one aligned evacuation copy serves both.
            # Split per head-pair so pair-1's v projection can be computed in
            # block (0,1) without false whole-tile dependencies.
            v_aug = [[perst.tile([P, 4, 2, P], FP16, name=f"vaug{pr}_{t}")
                      for t in range(NT)] for pr in range(NPAIR)]
            for pr in range(NPAIR):
                for t in range(NT):
                    nc.vector.memset(v_aug[pr][t][:, :, :, 0:64], 0.0)
                    nc.vector.memset(v_aug[pr][t][:, :, :, 0:1], 1.0)
            aoT = [[perst.tile([P, 512], BF16, name=f"aoT{p}_{t}")
                    for t in range(NT)] for p in range(NPAIR)]

            # ---------------- helpers ----------------
            def qk_chunk(ech, t, dest, cos_t, sin_t):
                # qkvT e-chunk [128, 512] = W-chunk @ xT-tile, then rotary.
                pqk = ps.tile([P, 512], F32, name="pqk", tag="m", bufs=2)
                for c in range(DC):
                    nc.tensor.matmul(pqk[:, :],
                                     wqk_chunk(ech, c),
                                     x_chunk(t, c),
                                     start=(c == 0), stop=(c == DC - 1))
                t1 = tmp.tile([P, 512], BF16, name="t1", tag="t1", bufs=2)
                t2 = tmp.tile([P, 512], BF16, name="t2", tag="t2", bufs=2)
                t3 = tmp.tile([P, 512], BF16, name="t3", tag="t3", bufs=2)
                nc.vector.tensor_tensor(t1[:, :], pqk[:, :], cos_t[t][:, :], op=MULT)
                nc.vector.tensor_tensor(t2[:, :], pqk[:, :], sin_t[t][:, :], op=MULT)
                nc.vector.stream_shuffle(t3[:, :], t2[:, :], PAIRSWAP)
                nc.vector.tensor_tensor(dest[:, :], t1[:, :], t3[:, :], op=ADD)

            def x_sub(t, c, nsl):
                if t == 0:
                    return xt0[c][:, nsl]
                return xtr[t - 1][c // 2][:, c % 2, nsl]

            def v_tile(t, pr):
                # v natural [n, e] for one head-pair, by 128-row subtiles
                for nsub in range(4):
                    pv = ps.tile([P, 128], F32, name="pv", tag="m", bufs=2)
                    for c in range(DC):
                        nc.tensor.matmul(pv[:, :],
                                         x_sub(t, c, slice(nsub * P, (nsub + 1) * P)),
                                         wv_sb[c // 4][:, c % 4, pr * 128:(pr + 1) * 128],
                                         start=(c == 0), stop=(c == DC - 1))
                    # ACT is idle during the warmup blocks (the only place
                    # v is computed); keep the DVE clear for the
                    # deadline-critical k rotary chains
                    nc.scalar.activation(
                        v_aug[pr][t][:, nsub, :, 64:128],
                        pv[:, :].rearrange("p (h d) -> p h d", h=2),
                        mybir.ActivationFunctionType.Copy)

            def qkv_for_tile(t, ops):
                for op in ops:
                    if op == "k0":
                        qk_chunk(2, t, krot[0][t], ckt, skt)
                    elif op == "k1":
                        qk_chunk(3, t, krot[1][t], ckt, skt)
                    elif op == "q0":
                        qk_chunk(0, t, qrot[0][t], cqt, sqt)
                    elif op == "q1":
                        qk_chunk(1, t, qrot[1][t], cqt, sqt)

            def attention(nq, pair, pre_jb=None, mid_jb=None):
                pav = [ps.tile([P, 512], F32, name=f"pav{h}", tag="av", bufs=2)
                       for h in range(2)]
                for jb in range(JB):
                    if pre_jb is not None:
                        pre_jb(jb)
                    sc = [ps.tile([P, 2, 512], F32, name=f"sc{h}", tag="s", bufs=2)
                          for h in range(2)]
                    for jl in range(2):
                        jt = jb * 2 + jl
                        kt = krot[pair][jt // 4]
                        jsl = slice((jt % 4) * P, (jt % 4 + 1) * P)
                        for h in range(2):
                            rows = slice(h * 64, (h + 1) * 64)
                            nc.tensor.matmul(sc[h][:, jl, :],
                                             kt[rows, jsl],
                                             qrot[pair][nq][rows, :],
                                             start=True, stop=True,
                                             tile_position=(h * 64, 0))
                    ex = [tmp.tile([P, 2, 512], FP16, name=f"ex{h}", tag="ex", bufs=6)
                          for h in range(2)]
                    for h in range(2):
                        nc.scalar.activation(ex[h][:, :, :], sc[h][:, :, :], EXP)
                    if mid_jb is not None:
                        mid_jb(jb)
                    for jl in range(2):
                        jt = jb * 2 + jl
                        for h in range(2):
                            nc.tensor.matmul(pav[h][:, :],
                                             v_aug[pair][jt // 4][:, jt % 4, h, :],
                                             ex[h][:, jl, :],
                                             start=(jt == 0), stop=(jt == JTILES - 1))

                def evac():
                    # evacuate psum (one copy: row 0 = denominators at
                    # base_partition 0 as the custom-DVE reciprocal needs,
                    # rows 64-127 = AV values), reciprocal, partition-broadcast,
                    # normalize into aoT
                    for h in range(2):
                        av_sb = tmp.tile([P, 512], F32, name="av_sb", tag="avs", bufs=2)
                        nc.vector.tensor_copy(av_sb[:, :], pav[h][:, :])
                        rc = tmp.tile([1, 512], F32, name="rc", tag="rc", bufs=2)
                        nc.vector.reciprocal_approx_fast(rc[:, :], av_sb[0:1, :])
                        bc = tmp.tile([P, 512], F32, name="bc", tag="bc", bufs=2)
                        if nq == NT - 1:
                            # tail-critical: broadcast via K=1 ones-matmul (no
                            # DMA round-trip before the last y projection)
                            rcr = tmp.tile([1, 512], BF16, name="rcr", tag="rcr", bufs=2)
                            nc.vector.tensor_copy(rcr[:, :], rc[:, :])
                            pbc = ps.tile([64, 512], F32, name="pbc", tag="m", bufs=2)
                            nc.tensor.matmul(pbc[:, :], ones_r[:, :], rcr[:, :],
                                             start=True, stop=True)
                            nc.vector.tensor_copy(bc[64:128, :], pbc[:, :])
                        else:
                            # broadcast across partitions via a DRAM round-trip
                            rd = dram.tile([1, 512], F32, name="rd", tag="rd", bufs=2)
                            nc.sync.dma_start(rd[:, :], rc[:, :])
                            nc.sync.dma_start(bc[64:128, :], rd.to_broadcast([64, 512]))
                        rows = slice(h * 64, (h + 1) * 64)
                        nc.vector.tensor_tensor(aoT[pair][nq][rows, :],
                                                av_sb[64:128, :], bc[64:128, :], op=MULT)

                evac()

            def y_proj_nsub(nq, nsub, out_d, row0):
                # both-pair projection for one 128-row query subtile
                ys = tmp.tile([P, DIM], FP16, name="ys", tag="ys", bufs=2)
                nsl = slice(nsub * P, (nsub + 1) * P)
                for dh2 in range(2):
                    py = ps.tile([P, 512], F32, name="py", tag="m", bufs=2)
                    dsl = slice(dh2 * 512, (dh2 + 1) * 512)
                    for pair in range(NPAIR):
                        nc.tensor.matmul(py[:, :],
                                         aoT[pair][nq][:, nsl],
                                         wo_sb[pair][dh2][:, :],
                                         start=(pair == 0), stop=(pair == NPAIR - 1))
                    nc.vector.tensor_copy(ys[:, dsl], py[:, :])
                nc.sync.dma_start(out_d[row0 + nsub * P:row0 + (nsub + 1) * P, :],
                                  ys[:, :])

            def y_proj_half(nq, half):
                for nsub in ((0, 1) if half == 0 else (2, 3)):
                    y_proj_nsub(nq, nsub, y_d, nq * 512)

            def y_proj_pair(nq, pair, out_d, row0):
                # single-pair partial projection (no cross-pair accumulation)
                for nsub in range(4):
                    ys = tmp.tile([P, DIM], FP16, name="ysp", tag="ys", bufs=2)
                    nsl = slice(nsub * P, (nsub + 1) * P)
                    for dh2 in range(2):
                        py = ps.tile([P, 512], F32, name="pyp", tag="m", bufs=2)
                        dsl = slice(dh2 * 512, (dh2 + 1) * 512)
                        nc.tensor.matmul(py[:, :], aoT[pair][nq][:, nsl],
                                         wo_sb[pair][dh2][:, :],
                                         start=True, stop=True)
                        nc.vector.tensor_copy(ys[:, dsl], py[:, :])
                    # 4-way split so the final (critical-path) writes drain at
                    # 4-queue parallelism instead of 11us on one queue
                    for q4 in range(4):
                        csl = slice(q4 * 256, (q4 + 1) * 256)
                        nc.sync.dma_start(
                            out_d[row0 + nsub * P:row0 + (nsub + 1) * P, csl],
                            ys[:, csl])

            # ---------------- emission order ----------------
            # Tile has sequential program-order semantics: every tile must be
            # written (in emission order) before anything that reads it, and
            # per-psum-tag slot reuse is FIFO in emission order. QKV work and
            # the output projections are threaded just-in-time through the
            # attention j-loops: k before the dots that need it, v between exp
            # and the AV that needs it, next-q early, y-projection halves into
            # BOTH pair blocks so the ACT-bound stretches keep the PE fed.
            # head: interleave the k0/q0 psum chains chunk-wise so both finish
            # as the last x/weight chunk's completion becomes visible, instead
            # of serially (the head is paced by DMA-completion processing)
            pqk_k = ps.tile([P, 512], F32, name="pqk", tag="m", bufs=2)
            pqk_q = ps.tile([P, 512], F32, name="pqk", tag="m", bufs=2)
            for c in range(DC):
                nc.tensor.matmul(pqk_k[:, :], wqk_chunk(2, c), x_chunk(0, c),
                                 start=(c == 0), stop=(c == DC - 1))
                nc.tensor.matmul(pqk_q[:, :], wqk_chunk(0, c), x_chunk(0, c),
                                 start=(c == 0), stop=(c == DC - 1))
            for psrc, dest, cs, sn in ((pqk_k, krot[0][0], ckt, skt),
                                       (pqk_q, qrot[0][0], cqt, sqt)):
                t1 = tmp.tile([P, 512], BF16, name="t1", tag="t1", bufs=2)
                t2 = tmp.tile([P, 512], BF16, name="t2", tag="t2", bufs=2)
                t3 = tmp.tile([P, 512], BF16, name="t3", tag="t3", bufs=2)
                nc.vector.tensor_tensor(t1[:, :], psrc[:, :], cs[0][:, :], op=MULT)
                nc.vector.tensor_tensor(t2[:, :], psrc[:, :], sn[0][:, :], op=MULT)
                nc.vector.stream_shuffle(t3[:, :], t2[:, :], PAIRSWAP)
                nc.vector.tensor_tensor(dest[:, :], t1[:, :], t3[:, :], op=ADD)
            # k0 for tile 1 right in the head: its matmuls overlap the first
            # dots/exp and the rotary lands well before the jb2 deadline
            qkv_for_tile(1, ["k0"])

            # JIT work is spread one chunk per j-batch with >=1 jb of slack
            # before its consumer, so the dots never queue behind a big lump
            # of projection matmuls in the PE FIFO.
            def pre_first(jb):
                ops = {1: ("k0", 2), 3: ("k0", 3), 4: ("k1", 0),
                       6: ("q1", 0)}
                if jb in ops:
                    op, t = ops[jb]
                    qkv_for_tile(t, [op])

            def mid_first(jb):
                if jb in (0, 2, 4, 6):
                    v_tile(jb // 2, 0)

            def pre01(jb):
                ops = {0: ("k1", 1), 1: ("q0", 1), 2: ("k1", 2),
                       3: ("q1", 1), 4: ("k1", 3)}
                if jb in ops:
                    op, t = ops[jb]
                    qkv_for_tile(t, [op])

            def mid01(jb):
                if jb in (0, 2, 4, 6):
                    v_tile(jb // 2, 1)

            def make_pre(nq, pair):
                def pre(jb):
                    if pair == 0 and nq + 1 < NT:
                        if jb == 1:
                            qkv_for_tile(nq + 1, ["q0"])
                        elif jb == 3:
                            qkv_for_tile(nq + 1, ["q1"])
                    if nq >= 1:
                        if jb == 4:
                            y_proj_nsub(nq - 1, 0 if pair == 0 else 2, y_d,
                                        (nq - 1) * 512)
                        elif jb == 6:
                            y_proj_nsub(nq - 1, 1 if pair == 0 else 3, y_d,
                                        (nq - 1) * 512)
                    if jb == 2 and (nq, pair) == (NT - 1, 1):
                        y_proj_pair(NT - 1, 0, y3a_d, 0)
                return pre

            for nq in range(NT):
                for pair in range(NPAIR):
                    if nq == 0 and pair == 0:
                        attention(nq, pair, pre_jb=pre_first, mid_jb=mid_first)
                    elif nq == 0 and pair == 1:
                        attention(nq, pair, pre_jb=pre01, mid_jb=mid01)
                    else:
                        attention(nq, pair, pre_jb=make_pre(nq, pair))
            y_proj_pair(NT - 1, 1, y_d, (NT - 1) * 512)
    nc.compile()
    return nc


def _host_prep(x, rotary_emb, w_qkv, w_out):
    """Build the 8 per-core input maps."""
    x = np.asarray(x, dtype=np.float32)
    rotary_emb = np.asarray(rotary_emb, dtype=np.float32)
    w_qkv = np.asarray(w_qkv, dtype=np.float32)
    w_out = np.asarray(w_out, dtype=np.float32)

    # interleaved dh permutation: new row 2i <- dim i, 2i+1 <- dim 32+i
    perm = np.empty(DH, dtype=np.int64)
    perm[0::2] = np.arange(32)
    perm[1::2] = np.arange(32) + 32
    pair_swap = np.arange(DH) ^ 1

    import ml_dtypes
    bf16 = ml_dtypes.bfloat16

    cos = np.cos(rotary_emb).T[perm]                      # [dh, n] permuted
    sin = np.sin(rotary_emb).T[perm]
    sign = np.where(perm < 32, -1.0, 1.0)[:, None].astype(np.float32)
    sin_eff = sign * sin
    sin_pre = sin_eff[pair_swap]                          # pre-swapped
    c2 = np.concatenate([cos, cos], axis=0)               # [128, n]
    s2 = np.concatenate([sin_pre, sin_pre], axis=0)
    cq = np.ascontiguousarray((SCALE * c2).astype(bf16))
    sq = np.ascontiguousarray((SCALE * s2).astype(bf16))
    ck = np.ascontiguousarray(c2.astype(bf16))
    sk = np.ascontiguousarray(s2.astype(bf16))

    in_maps = []
    for core in range(NCORES):
        b = core // (NCORES // B)
        g = core % (NCORES // B)
        heads = range(4 * g, 4 * g + HPC)
        q_rows = np.concatenate([h * DH + perm for h in heads])
        k_rows = np.concatenate([INNER + h * DH + perm for h in heads])
        v_rows = np.arange(2 * INNER + 4 * g * DH, 2 * INNER + (4 * g + HPC) * DH)
        wqkT = np.ascontiguousarray(w_qkv[np.concatenate([q_rows, k_rows])].T.astype(bf16))
        wvT = np.ascontiguousarray(w_qkv[v_rows].T.astype(bf16))
        woT = np.ascontiguousarray(w_out[:, 4 * g * DH:(4 * g + HPC) * DH].T.astype(bf16))
        xT = np.ascontiguousarray(x[b].T.astype(bf16))
        in_maps.append({
            "xT": xT, "wqkT": wqkT, "wvT": wvT,
            "cq": cq, "sq": sq, "ck": ck, "sk": sk, "woT": woT,
        })
    return in_maps


def kernel(x, rotary_emb, w_qkv, w_out, b_out, _trace=False):
    if "nc" not in _CACHE:
        _CACHE["nc"] = _build()
    nc = _CACHE["nc"]
    in_maps = _host_prep(x, rotary_emb, w_qkv, w_out)
    res = run_bass_kernel_spmd(nc, in_maps, core_ids=list(range(NCORES)),
                               trace=_trace)
    _CACHE["last_result"] = res
    y = np.zeros((B, N, DIM), dtype=np.float32)
    for core in range(NCORES):
        b = core // (NCORES // B)
        y[b] += res.results[core]["y"]
        y[b, (NT - 1) * 512:] += res.results[core]["y3a"]
    y += np.asarray(b_out, dtype=np.float32)[None, None, :]
    return y


# revision 40
# speedup vs baseline: 1.0122x; 1.0122x over previous
"""Multi-head attention (QKV proj + rotary + softmax attention + out proj)
for Trainium2, sharded over 8 NeuronCores.

Problem: x[2,2048,1024], 16 heads x dh=64, rotary embedding, softmax
attention, output projection + bias.

Sharding: batch x head-group. Core c handles batch c//4 and the 4 heads
[4*(c%4), 4*(c%4)+4). Each core computes its QKV slice, rotary, attention,
and a partial output projection; the host sums the 4 partial projections
per batch and adds the bias.

Device-side design (per core, everything in "transposed" layout):
  - all inputs (x, weights, cos/sin) converted to bf16 on the host: halves
    HBM traffic and keeps every matmul at the full 1-cycle/row PE rate.
    Measured rel err ~7e-3 (gate 2e-2).
  - qkvT = W @ x^T as bf16 matmuls: qT/kT produced as [dh-pair(128), n]
    tiles, v as natural [n, e] tiles, one head-pair at a time.
  - rotary applied on the fp32 psum output via DVE: q*cos +
    pairswap(q*sin_pre), with the dh dimension stored interleaved
    ([0,32,1,33,...]) so rotate_half becomes an adjacent-lane
    stream_shuffle. Output bf16.
  - dots: scoresT[j,n] = krotT^T-slice @ qrotT, two heads packed in the
    128x128 PE array via tile_position row-tiling (K=64 each, concurrent).
  - softmax without max-subtraction (logits are O(+-6)): ACT exp over
    2-j-tile psum batches (N=1024 per ACTIVATE), output fp16. The exp
    stream (128 ACTIVATEs, ~142us) is the kernel's critical resource;
    everything else is scheduled to hide under it.
  - AV: lhsT = [ones | zeros(63) | v(64)] (M=128, fp16) so psum row 0
    accumulates the softmax denominators (base_partition 0 for the DVE
    reciprocal) and values land at rows 64-127 (32-aligned); fp32 psum
    accumulation over the 16 j-tiles, one aligned evacuation copy.
  - normalize: reciprocal_approx_fast of the sums row, partition-broadcast
    via a DRAM round-trip DMA (K=1 ones-matmul on the tail block), one DVE
    multiply -> aoT (bf16).
  - output proj: y[n,d] accumulated over the two head-pair e-chunks, fp16
    out; host sums the per-core partials in fp32 and adds the bias.

Scheduling (the measured-trace-driven part):
  - DMA-completion processing serializes at ~0.67us/DMA on the Sync
    engine, so the preamble uses few, need-ordered dma_starts: first-needed
    tiles small and early, later ones fat. All of x (bf16, 4MB) is
    prefetched once and stays resident.
  - the head interleaves the k0/q0 psum chains chunk-wise so the first
    dots fire as soon as the last x chunk's completion is visible.
  - per-tile k projections, next-tile q rotations and per-128-row y
    projections are threaded through the attention j-loops one chunk per
    j-batch with >=1 batch of slack before their consumer, so the dots
    (which gate exp) never queue behind a lump of projection matmuls in
    the PE FIFO. Block (0,0) computes only pair-0's k/v; block (0,1)
    computes pair-1's, balancing both warmup blocks against the exp
    stream and keeping the PE HAM window at K=8/8 throughout.
"""
import sys

sys.path.insert(0, "/opt/trn_rl_repo")

import numpy as np

import concourse.bacc as bacc
import concourse.tile as tile
from concourse import mybir
from concourse.bass_utils import run_bass_kernel_spmd

F32 = mybir.dt.float32
F32R = mybir.dt.float32r
BF16 = mybir.dt.bfloat16
FP16 = mybir.dt.float16
EXP = mybir.ActivationFunctionType.Exp
MULT = mybir.AluOpType.mult
ADD = mybir.AluOpType.add

B, N, DIM = 2, 2048, 1024
H, DH = 16, 64
INNER = H * DH
SCALE = DH ** -0.5
NCORES = 8
HPC = H // (NCORES // B)      # heads per core = 4
NPAIR = HPC // 2              # head pairs per core = 2

P = 128
NT = N // 512                 # 4 n-tiles of 512
DC = DIM // P                 # 8 d-chunks
JTILES = N // P               # 16 j-tiles
JB = JTILES // 2              # 8 j-batches (2 j-tiles each)

PAIRSWAP = [i ^ 1 for i in range(32)]

_CACHE = {}


def _build():
    nc = bacc.Bacc(None, target_bir_lowering=False, debug=False)
    with tile.TileContext(nc) as tc:
        with tc.tile_pool(name="dram", bufs=1, space="DRAM") as dram, \
             tc.tile_pool(name="const", bufs=1) as const, \
             tc.tile_pool(name="perst", bufs=1) as perst, \
             tc.tile_pool(name="tmp", bufs=1) as tmp, \
             tc.tile_pool(name="ps", bufs=1, space="PSUM") as ps:
            # ---------------- DRAM I/O ----------------
            xT_d = dram.tile([DIM, N], BF16, kind="ExternalInput", name="xT", uniquify=False)
            wqkT_d = dram.tile([DIM, 512], BF16, kind="ExternalInput", name="wqkT", uniquify=False)
            wvT_d = dram.tile([DIM, 256], BF16, kind="ExternalInput", name="wvT", uniquify=False)
            cq_d = dram.tile([P, N], BF16, kind="ExternalInput", name="cq", uniquify=False)
            sq_d = dram.tile([P, N], BF16, kind="ExternalInput", name="sq", uniquify=False)
            ck_d = dram.tile([P, N], BF16, kind="ExternalInput", name="ck", uniquify=False)
            sk_d = dram.tile([P, N], BF16, kind="ExternalInput", name="sk", uniquify=False)
            woT_d = dram.tile([256, DIM], BF16, kind="ExternalInput", name="woT", uniquify=False)
            y_d = dram.tile([N, DIM], FP16, kind="ExternalOutput", name="y", uniquify=False)
            y3a_d = dram.tile([512, DIM], FP16, kind="ExternalOutput", name="y3a", uniquify=False)

            xT_r = xT_d.rearrange("(c p) n -> p c n", p=P)
            wqk_r = wqkT_d.rearrange("(c p) e -> p c e", p=P)
            wv_r = wvT_d.rearrange("(c p) e -> p c e", p=P)
            wo_r = woT_d.rearrange("(c p) d -> p c d", p=P)

            # ------------- const tiles -------------
            # DMA-completion processing serializes at ~0.67us each on the Sync
            # engine, so the preamble uses as FEW dma_starts as data-arrival
            # deadlines allow: first-needed tiles small and early, the rest fat.
            # e-chunk order in wqkT columns: q0=0, q1=1, k0=2, k1=3 (x128)
            wqk_sb = [[const.tile([P, 4, P], BF16, name=f"wqk{e}_{hf}")
                       for hf in range(2)] for e in range(4)]
            wv_sb = [const.tile([P, 4, 256], BF16, name=f"wv{hf}")
                     for hf in range(2)]
            wo_sb = [[const.tile([P, 512], BF16, name=f"wo{p}_{d}")
                      for d in range(2)] for p in range(NPAIR)]
            cqt = [const.tile([P, 512], BF16, name=f"cqt{t}") for t in range(NT)]
            sqt = [const.tile([P, 512], BF16, name=f"sqt{t}") for t in range(NT)]
            ckt = [const.tile([P, 512], BF16, name=f"ckt{t}") for t in range(NT)]
            skt = [const.tile([P, 512], BF16, name=f"skt{t}") for t in range(NT)]

            def wqk_chunk(e, c):
                return wqk_sb[e][c // 4][:, c % 4, :]

            def dma_wqk(e, hf):
                nc.sync.dma_start(
                    wqk_sb[e][hf][:, :, :],
                    wqk_r[:, hf * 4:(hf + 1) * 4, e * P:(e + 1) * P])

            def dma_cs(tiles, src_d, t):
                nc.sync.dma_start(tiles[t][:, :], src_d[:, t * 512:(t + 1) * 512])

            # all x tiles prefetched in the preamble (bf16: 4MB total) and
            # kept resident for the whole kernel -- x is loaded exactly once.
            # t0 is chunk-granular (feeds the very first matmuls); t1-3 use
            # c-pair tiles (half the dma_starts, deadlines are later).
            xt0 = [tmp.tile([P, 512], BF16, name=f"xt0_{c}", tag=f"xt{c}",
                            bufs=1) for c in range(DC)]
            xtr = [[tmp.tile([P, 2, 512], BF16, name=f"xt{t}_{cp}",
                             tag=f"xtp{cp}", bufs=3) for cp in range(DC // 2)]
                   for t in range(1, NT)]

            def x_chunk(t, c):
                if t == 0:
                    return xt0[c][:, :]
                return xtr[t - 1][c // 2][:, c % 2, :]

            def dma_x(t):
                for cp in range(DC // 2):
                    nc.sync.dma_start(
                        xtr[t - 1][cp][:, :, :],
                        xT_r[:, 2 * cp:2 * cp + 2, t * 512:(t + 1) * 512])

            # ---------------- preamble emission (need-order) ----------------
            for hf in range(2):
                dma_wqk(2, hf)         # k0
            for c in range(DC):
                nc.sync.dma_start(xt0[c][:, :], xT_r[:, c, 0:512])
            dma_cs(ckt, ck_d, 0)
            dma_cs(skt, sk_d, 0)
            for hf in range(2):
                dma_wqk(0, hf)         # q0
            dma_cs(cqt, cq_d, 0)
            dma_cs(sqt, sq_d, 0)
            dma_cs(ckt, ck_d, 1)
            dma_cs(skt, sk_d, 1)
            dma_x(1)
            for hf in range(2):
                nc.sync.dma_start(wv_sb[hf][:, :, :],
                                  wv_r[:, hf * 4:(hf + 1) * 4, :])
            for hf in range(2):
                dma_wqk(3, hf)         # k1
            dma_cs(ckt, ck_d, 2)
            dma_cs(skt, sk_d, 2)
            for hf in range(2):
                dma_wqk(1, hf)         # q1
            dma_cs(ckt, ck_d, 3)
            dma_cs(skt, sk_d, 3)
            dma_x(2)
            dma_x(3)
            for t in range(1, NT):
                dma_cs(cqt, cq_d, t)
                dma_cs(sqt, sq_d, t)
            for p in range(NPAIR):
                for d in range(2):
                    nc.sync.dma_start(wo_sb[p][d][:, :], wo_r[:, p, d * 512:(d + 1) * 512])

            ones_f = const.tile([1, 64], F32)
            nc.vector.memset(ones_f[:, :], 1.0)
            ones_r = const.tile([1, 64], BF16)
            nc.vector.tensor_copy(ones_r[:, :], ones_f[:, :])

            # ---------------- persistent tiles ----------------
            qrot = [[perst.tile([P, 512], BF16, name=f"qrot{p}_{t}")
                     for t in range(NT)] for p in range(NPAIR)]
            krot = [[perst.tile([P, 512], BF16, name=f"krot{p}_{t}")
                     for t in range(NT)] for p in range(NPAIR)]
            # AV stationary operand is [ones | zeros(63) | v(64)] so psum row 0
            # accumulates the softmax denominators (base_partition 0 for the
            # DVE reciprocal) and the values land at rows 64-127 (32-aligned
            # partition base) -- one aligned evacuation copy serves both.
            # Split per head-pair so pair-1's v projection can be computed in
            # block (0,1) without false whole-tile dependencies.
            v_aug = [[perst.tile([P, 4, 2, P], FP16, name=f"vaug{pr}_{t}")
                      for t in range(NT)] for pr in range(NPAIR)]
            for pr in range(NPAIR):
                for t in range(NT):
                    nc.vector.memset(v_aug[pr][t][:, :, :, 0:64], 0.0)
                    nc.vector.memset(v_aug[pr][t][:, :, :, 0:1], 1.0)
            aoT = [[perst.tile([P, 512], BF16, name=f"aoT{p}_{t}")
                    for t in range(NT)] for p in range(NPAIR)]

            # ---------------- helpers ----------------
            def qk_chunk(ech, t, dest, cos_t, sin_t):
                # qkvT e-chunk [128, 512] = W-chunk @ xT-tile, then rotary.
                pqk = ps.tile([P, 512], F32, name="pqk", tag="m", bufs=2)
                for c in range(DC):
                    nc.tensor.matmul(pqk[:, :],
                                     wqk_chunk(ech, c),
                                     x_chunk(t, c),
                                     start=(c == 0), stop=(c == DC - 1))
                t1 = tmp.tile([P, 512], BF16, name="t1", tag="t1", bufs=2)
                t2 = tmp.tile([P, 512], BF16, name="t2", tag="t2", bufs=2)
                t3 = tmp.tile([P, 512], BF16, name="t3", tag="t3", bufs=2)
                nc.vector.tensor_tensor(t1[:, :], pqk[:, :], cos_t[t][:, :], op=MULT)
                nc.vector.tensor_tensor(t2[:, :], pqk[:, :], sin_t[t][:, :], op=MULT)
                nc.vector.stream_shuffle(t3[:, :], t2[:, :], PAIRSWAP)
                nc.vector.tensor_tensor(dest[:, :], t1[:, :], t3[:, :], op=ADD)

            def x_sub(t, c, nsl):
                if t == 0:
                    return xt0[c][:, nsl]
                return xtr[t - 1][c // 2][:, c % 2, nsl]

            def v_tile(t, pr):
                # v natural [n, e] for one head-pair, by 128-row subtiles
                for nsub in range(4):
                    pv = ps.tile([P, 128], F32, name="pv", tag="m", bufs=2)
                    for c in range(DC):
                        nc.tensor.matmul(pv[:, :],
                                         x_sub(t, c, slice(nsub * P, (nsub + 1) * P)),
                                         wv_sb[c // 4][:, c % 4, pr * 128:(pr + 1) * 128],
                                         start=(c == 0), stop=(c == DC - 1))
                    # ACT is idle during the warmup blocks (the only place
                    # v is computed); keep the DVE clear for the
                    # deadline-critical k rotary chains
                    nc.scalar.activation(
                        v_aug[pr][t][:, nsub, :, 64:128],
                        pv[:, :].rearrange("p (h d) -> p h d", h=2),
                        mybir.ActivationFunctionType.Copy)

            def qkv_for_tile(t, ops):
                for op in ops:
                    if op == "k0":
                        qk_chunk(2, t, krot[0][t], ckt, skt)
                    elif op == "k1":
                        qk_chunk(3, t, krot[1][t], ckt, skt)
                    elif op == "q0":
                        qk_chunk(0, t, qrot[0][t], cqt, sqt)
                    elif op == "q1":
                        qk_chunk(1, t, qrot[1][t], cqt, sqt)

            def attention(nq, pair, pre_jb=None, mid_jb=None):
                pav = [ps.tile([P, 512], F32, name=f"pav{h}", tag="av", bufs=2)
                       for h in range(2)]
                for jb in range(JB):
                    if pre_jb is not None:
                        pre_jb(jb)
                    sc = [ps.tile([P, 2, 512], F32, name=f"sc{h}", tag="s", bufs=2)
                          for h in range(2)]
                    for jl in range(2):
                        jt = jb * 2 + jl
                        kt = krot[pair][jt // 4]
                        jsl = slice((jt % 4) * P, (jt % 4 + 1) * P)
                        for h in range(2):
                            rows = slice(h * 64, (h + 1) * 64)
                            nc.tensor.matmul(sc[h][:, jl, :],
                                             kt[rows, jsl],
                                             qrot[pair][nq][rows, :],
                                             start=True, stop=True,
                                             tile_position=(h * 64, 0))
                    ex = [tmp.tile([P, 2, 512], FP16, name=f"ex{h}", tag="ex", bufs=6)
                          for h in range(2)]
                    for h in range(2):
                        nc.scalar.activation(ex[h][:, :, :], sc[h][:, :, :], EXP)
                    if mid_jb is not None:
                        mid_jb(jb)
                    for jl in range(2):
                        jt = jb * 2 + jl
                        for h in range(2):
                            nc.tensor.matmul(pav[h][:, :],
                                             v_aug[pair][jt // 4][:, jt % 4, h, :],
                                             ex[h][:, jl, :],
                                             start=(jt == 0), stop=(jt == JTILES - 1))

                def evac():
                    # evacuate psum (one copy: row 0 = denominators at
                    # base_partition 0 as the custom-DVE reciprocal needs,
                    # rows 64-127 = AV values), reciprocal, partition-broadcast,
                    # normalize into aoT
                    for h in range(2):
                        av_sb = tmp.tile([P, 512], F32, name="av_sb", tag="avs", bufs=2)
                        nc.vector.tensor_copy(av_sb[:, :], pav[h][:, :])
                        rc = tmp.tile([1, 512], F32, name="rc", tag="rc", bufs=2)
                        nc.vector.reciprocal_approx_fast(rc[:, :], av_sb[0:1, :])
                        bc = tmp.tile([P, 512], F32, name="bc", tag="bc", bufs=2)
                        if nq == NT - 1:
                            # tail-critical: broadcast via K=1 ones-matmul (no
                            # DMA round-trip before the last y projection)
                            rcr = tmp.tile([1, 512], BF16, name="rcr", tag="rcr", bufs=2)
                            nc.vector.tensor_copy(rcr[:, :], rc[:, :])
                            pbc = ps.tile([64, 512], F32, name="pbc", tag="m", bufs=2)
                            nc.tensor.matmul(pbc[:, :], ones_r[:, :], rcr[:, :],
                                             start=True, stop=True)
                            nc.vector.tensor_copy(bc[64:128, :], pbc[:, :])
                        else:
                            # broadcast across partitions via a DRAM round-trip
                            rd = dram.tile([1, 512], F32, name="rd", tag="rd", bufs=2)
                            nc.sync.dma_start(rd[:, :], rc[:, :])
                            nc.sync.dma_start(bc[64:128, :], rd.to_broadcast([64, 512]))
                        rows = slice(h * 64, (h + 1) * 64)
                        nc.vector.tensor_tensor(aoT[pair][nq][rows, :],
                                                av_sb[64:128, :], bc[64:128, :], op=MULT)

                evac()

            def y_proj_nsub(nq, nsub, out_d, row0):
                # both-pair projection for one 128-row query subtile
                ys = tmp.tile([P, DIM], FP16, name="ys", tag="ys", bufs=2)
                nsl = slice(nsub * P, (nsub + 1) * P)
                for dh2 in range(2):
                    py = ps.tile([P, 512], F32, name="py", tag="m", bufs=2)
                    dsl = slice(dh2 * 512, (dh2 + 1) * 512)
                    for pair in range(NPAIR):
                        nc.tensor.matmul(py[:, :],
                                         aoT[pair][nq][:, nsl],
                                         wo_sb[pair][dh2][:, :],
                                         start=(pair == 0), stop=(pair == NPAIR - 1))
                    nc.vector.tensor_copy(ys[:, dsl], py[:, :])
                nc.sync.dma_start(out_d[row0 + nsub * P:row0 + (nsub + 1) * P, :],
                                  ys[:, :])

            def y_proj_half(nq, half):
                for nsub in ((0, 1) if half == 0 else (2, 3)):
                    y_proj_nsub(nq, nsub, y_d, nq * 512)

            def y_proj_pair(nq, pair, out_d, row0):
                # single-pair partial projection (no cross-pair accumulation).
                # Per-dh2-half staging tiles: each half's (critical-path tail)
                # DMA starts right after its own copy and the two halves drain
                # on different queues -- fewer serialized DMA completions than
                # a finer split, earlier start than a monolithic write.
                for nsub in range(4):
                    nsl = slice(nsub * P, (nsub + 1) * P)
                    for dh2 in range(2):
                        ys = tmp.tile([P, 512], FP16, name="ysp", tag="ysp", bufs=6)
                        py = ps.tile([P, 512], F32, name="pyp", tag="m", bufs=2)
                        dsl = slice(dh2 * 512, (dh2 + 1) * 512)
                        nc.tensor.matmul(py[:, :], aoT[pair][nq][:, nsl],
                                         wo_sb[pair][dh2][:, :],
                                         start=True, stop=True)
                        nc.vector.tensor_copy(ys[:, :], py[:, :])
                        nc.sync.dma_start(
                            out_d[row0 + nsub * P:row0 + (nsub + 1) * P, dsl],
                            ys[:, :])

            # ---------------- emission order ----------------
            # Tile has sequential program-order semantics: every tile must be
            # written (in emission order) before anything that reads it, and
            # per-psum-tag slot reuse is FIFO in emission order. QKV work and
            # the output projections are threaded just-in-time through the
            # attention j-loops: k before the dots that need it, v between exp
            # and the AV that needs it, next-q early, y-projection halves into
            # BOTH pair blocks so the ACT-bound stretches keep the PE fed.
            # head: interleave the k0/q0 psum chains chunk-wise so both finish
            # as the last x/weight chunk's completion becomes visible, instead
            # of serially (the head is paced by DMA-completion processing)
            pqk_k = ps.tile([P, 512], F32, name="pqk", tag="m", bufs=2)
            pqk_q = ps.tile([P, 512], F32, name="pqk", tag="m", bufs=2)
            for c in range(DC):
                nc.tensor.matmul(pqk_k[:, :], wqk_chunk(2, c), x_chunk(0, c),
                                 start=(c == 0), stop=(c == DC - 1))
                nc.tensor.matmul(pqk_q[:, :], wqk_chunk(0, c), x_chunk(0, c),
                                 start=(c == 0), stop=(c == DC - 1))
            for psrc, dest, cs, sn in ((pqk_k, krot[0][0], ckt, skt),
                                       (pqk_q, qrot[0][0], cqt, sqt)):
                t1 = tmp.tile([P, 512], BF16, name="t1", tag="t1", bufs=2)
                t2 = tmp.tile([P, 512], BF16, name="t2", tag="t2", bufs=2)
                t3 = tmp.tile([P, 512], BF16, name="t3", tag="t3", bufs=2)
                nc.vector.tensor_tensor(t1[:, :], psrc[:, :], cs[0][:, :], op=MULT)
                nc.vector.tensor_tensor(t2[:, :], psrc[:, :], sn[0][:, :], op=MULT)
                nc.vector.stream_shuffle(t3[:, :], t2[:, :], PAIRSWAP)
                nc.vector.tensor_tensor(dest[:, :], t1[:, :], t3[:, :], op=ADD)
            # k0 for tile 1 right in the head: its matmuls overlap the first
            # dots/exp and the rotary lands well before the jb2 deadline
            qkv_for_tile(1, ["k0"])

            # JIT work is spread one chunk per j-batch with >=1 jb of slack
            # before its consumer, so the dots never queue behind a big lump
            # of projection matmuls in the PE FIFO.
            def pre_first(jb):
                ops = {1: ("k0", 2), 3: ("k0", 3), 4: ("k1", 0),
                       6: ("q1", 0)}
                if jb in ops:
                    op, t = ops[jb]
                    qkv_for_tile(t, [op])

            def mid_first(jb):
                if jb in (0, 2, 4, 6):
                    v_tile(jb // 2, 0)

            def pre01(jb):
                ops = {0: ("k1", 1), 1: ("q0", 1), 2: ("k1", 2),
                       3: ("q1", 1), 4: ("k1", 3)}
                if jb in ops:
                    op, t = ops[jb]
                    qkv_for_tile(t, [op])

            def mid01(jb):
                if jb in (0, 2, 4, 6):
                    v_tile(jb // 2, 1)

            def make_pre(nq, pair):
                def pre(jb):
                    if pair == 0 and nq + 1 < NT:
                        if jb == 1:
                            qkv_for_tile(nq + 1, ["q0"])
                        elif jb == 3:
                            qkv_for_tile(nq + 1, ["q1"])
                    if nq >= 1:
                        if jb == 4:
                            y_proj_nsub(nq - 1, 0 if pair == 0 else 2, y_d,
                                        (nq - 1) * 512)
                        elif jb == 6:
                            y_proj_nsub(nq - 1, 1 if pair == 0 else 3, y_d,
                                        (nq - 1) * 512)
                    if jb == 2 and (nq, pair) == (NT - 1, 1):
                        y_proj_pair(NT - 1, 0, y3a_d, 0)
                return pre

            for nq in range(NT):
                for pair in range(NPAIR):
                    if nq == 0 and pair == 0:
                        attention(nq, pair, pre_jb=pre_first, mid_jb=mid_first)
                    elif nq == 0 and pair == 1:
                        attention(nq, pair, pre_jb=pre01, mid_jb=mid01)
                    else:
                        attention(nq, pair, pre_jb=make_pre(nq, pair))
            y_proj_pair(NT - 1, 1, y_d, (NT - 1) * 512)
    nc.compile()
    return nc


def _host_prep(x, rotary_emb, w_qkv, w_out):
    """Build the 8 per-core input maps."""
    x = np.asarray(x, dtype=np.float32)
    rotary_emb = np.asarray(rotary_emb, dtype=np.float32)
    w_qkv = np.asarray(w_qkv, dtype=np.float32)
    w_out = np.asarray(w_out, dtype=np.float32)

    # interleaved dh permutation: new row 2i <- dim i, 2i+1 <- dim 32+i
    perm = np.empty(DH, dtype=np.int64)
    perm[0::2] = np.arange(32)
    perm[1::2] = np.arange(32) + 32
    pair_swap = np.arange(DH) ^ 1

    import ml_dtypes
    bf16 = ml_dtypes.bfloat16

    cos = np.cos(rotary_emb).T[perm]                      # [dh, n] permuted
    sin = np.sin(rotary_emb).T[perm]
    sign = np.where(perm < 32, -1.0, 1.0)[:, None].astype(np.float32)
    sin_eff = sign * sin
    sin_pre = sin_eff[pair_swap]                          # pre-swapped
    c2 = np.concatenate([cos, cos], axis=0)               # [128, n]
    s2 = np.concatenate([sin_pre, sin_pre], axis=0)
    cq = np.ascontiguousarray((SCALE * c2).astype(bf16))
    sq = np.ascontiguousarray((SCALE * s2).astype(bf16))
    ck = np.ascontiguousarray(c2.astype(bf16))
    sk = np.ascontiguousarray(s2.astype(bf16))

    in_maps = []
    for core in range(NCORES):
        b = core // (NCORES // B)
        g = core % (NCORES // B)
        heads = range(4 * g, 4 * g + HPC)
        q_rows = np.concatenate([h * DH + perm for h in heads])
        k_rows = np.concatenate([INNER + h * DH + perm for h in heads])
        v_rows = np.arange(2 * INNER + 4 * g * DH, 2 * INNER + (4 * g + HPC) * DH)
        wqkT = np.ascontiguousarray(w_qkv[np.concatenate([q_rows, k_rows])].T.astype(bf16))
        wvT = np.ascontiguousarray(w_qkv[v_rows].T.astype(bf16))
        woT = np.ascontiguousarray(w_out[:, 4 * g * DH:(4 * g + HPC) * DH].T.astype(bf16))
        xT = np.ascontiguousarray(x[b].T.astype(bf16))
        in_maps.append({
            "xT": xT, "wqkT": wqkT, "wvT": wvT,
            "cq": cq, "sq": sq, "ck": ck, "sk": sk, "woT": woT,
        })
    return in_maps


def kernel(x, rotary_emb, w_qkv, w_out, b_out, _trace=False):
    if "nc" not in _CACHE:
        _CACHE["nc"] = _build()
    nc = _CACHE["nc"]
    in_maps = _host_prep(x, rotary_emb, w_qkv, w_out)
    res = run_bass_kernel_spmd(nc, in_maps, core_ids=list(range(NCORES)),
                               trace=_trace)
    _CACHE["last_result"] = res
    y = np.zeros((B, N, DIM), dtype=np.float32)
    for core in range(NCORES):
        b = core // (NCORES // B)
        y[b] += res.results[core]["y"]
        y[b, (NT - 1) * 512:] += res.results[core]["y3a"]
    y += np.asarray(b_out, dtype=np.float32)[None, None, :]
    return y
